# revision 33
# baseline (speedup 1.0000x reference)
"""Trainium2 Bass kernel for nn_CritiGraph.

Math (vs the fp32 reference):
  dist(c1,c2,n) = sg * (1 - e/16) * n,  sg = sign(c1)*sign(c2),
  e = frexp_exp(|c1|^|c2| + 1) = bexp(float(|c1|^|c2|) + 1.5) - 126.
  ct[t,s,c,tp] = Q[t,s,tp] + M1[t,s,tp] * R[t,s,c,tp]
  where R = sgc * (e-16) (cnc sign applied via bf16 sign-bit xor),
        M1 = -sgp * norm / 128  (pos sign + /TP/16 folded),
        Q  = (sum_tp g - g)/8,  g = cos_sta_pos.

v2 layout: everything in natural (c,tp) order, S=128 on partitions.
Main loop per t (all full-width 2056-elem ops):
  p1  DVE TT : z16 = cmagR ^ pmag[s,tp] (bcast along c, stride-0 free dim)
  p2  ACT    : w = fp32(z16) + 1.5
  p3  DVE TS : sa = bf16((bits(w) >> 23) - 142)       [= e - 16, signed]
  p4  DVE TT : r16 = bits(sa) ^ sgn_mask (0x8000/0)
  p5  split by tp lane:
      lanes [0,NS):  ACT affine  out = M1*r + Q   (per-partition scale/bias)
      lanes [NS,8):  DVE TT r2 = r + Q2 (Q2 = Q/M1, bcast along c)
                     DVE TT out = r2 * M1 (bcast along c)
Sharding: T=128 rows split across 8 cores (16 rows each). Output gathered
on host (and upcast to fp32 if OUT_DT is bf16).
"""
import dataclasses
import numpy as np

import concourse.bass as bass
import concourse.mybir as mybir
from concourse import tile, bacc
from concourse.bass_utils import run_bass_kernel_spmd

dt = mybir.dt
Alu = mybir.AluOpType
Act = mybir.ActivationFunctionType

T, S, TP, C = 128, 128, 8, 257
NCORES = 8
TL = T // NCORES          # 16 t-rows per core
FD = C * TP               # 2056 free width, (c,tp) natural order

NS = 2                    # tp lanes [0,NS) handled by ACT affine slices
OUT_DT = "bf16"           # 'f32' | 'bf16' (bf16 is upcast to f32 on host)


def _rep128(ap_row):
    """DRAM row AP -> same row broadcast to 128 partitions (stride-0)."""
    return dataclasses.replace(ap_row, ap=[[0, 128]] + list(ap_row.ap)[1:])


def _sub(ap_flat, off_elems, dims):
    """Offset a flat [P, N] AP by off_elems and install custom free dims."""
    s = ap_flat[:, off_elems:off_elems + 1]
    return dataclasses.replace(s, ap=[list(s.ap)[0]] + [list(d) for d in dims])


def build_nc(ns=None, out_dt=None):
    ns = NS if ns is None else ns
    out_dt = OUT_DT if out_dt is None else out_dt
    odt = dt.float32 if out_dt == "f32" else dt.bfloat16
    nc = bacc.Bacc("TRN2", target_bir_lowering=False, debug=False)

    sta_d = nc.dram_tensor("sta_loc", [TL, TP], dt.int32, kind="ExternalInput")
    pos_d = nc.dram_tensor("pos_loc", [TL, S, TP], dt.int32, kind="ExternalInput")
    cnc_d = nc.dram_tensor("cnc_loc", [TL, C, TP], dt.int32, kind="ExternalInput")
    norm_d = nc.dram_tensor("eu_norm", [TL, S], dt.float32, kind="ExternalInput")
    ct_d = nc.dram_tensor("ct", [TL, S, C, TP], odt, kind="ExternalOutput")

    with tile.TileContext(nc) as tc:
        with (
            tc.tile_pool(name="const", bufs=1) as cpool,
            tc.tile_pool(name="work", bufs=4) as wpool,
            tc.tile_pool(name="dram", bufs=1, space="DRAM") as dpool,
        ):
            # ---------------- preprocessing (small) ----------------
            # cnc -> (c,tp) magnitude / sign-mask planes, staged to DRAM
            cncraw = cpool.tile([TL, FD], dt.int32)
            nc.sync.dma_start(cncraw[:], cnc_d[:].rearrange("t c p -> t (c p)"))
            c_mag16 = cpool.tile([TL, FD], dt.uint16)
            c_sg0 = cpool.tile([TL, FD], dt.bfloat16)
            c_sgn = cpool.tile([TL, FD], dt.bfloat16)
            nc.scalar.activation(c_mag16[:], cncraw[:], Act.Abs)
            # sign as +-1 bf16: (cnc < 0) * -2 + 1
            nc.vector.tensor_scalar(c_sg0[:], cncraw[:], 0.0, -2.0,
                                    Alu.is_lt, Alu.mult)
            nc.vector.tensor_scalar(c_sgn[:], c_sg0[:], 1.0, None, Alu.add)
            d_cmag = dpool.tile([TL, FD], dt.uint16)
            d_csgn = dpool.tile([TL, FD], dt.bfloat16)
            nc.sync.dma_start(d_cmag[:], c_mag16[:])
            nc.sync.dma_start(d_csgn[:], c_sgn[:])

            # pos: [s, (t,tp)] in one strided DMA
            posraw = cpool.tile([S, TL * TP], dt.int32)
            pos_src = dataclasses.replace(
                pos_d[:].rearrange("t s p -> t (s p)").flatten(),
                ap=[[TP, S], [S * TP, TL], [1, TP]])
            nc.sync.dma_start(
                posraw[:].rearrange("s (t p) -> s t p", p=TP), pos_src)
            p_mag16 = cpool.tile([S, TL * TP], dt.uint16)
            nc.scalar.activation(p_mag16[:], posraw[:], Act.Abs)

            # norm[s, t] via strided DMA (4B gather, 8KB once)
            norm_sb = cpool.tile([S, TL], dt.float32)
            norm_src = dataclasses.replace(
                norm_d[:].flatten(), ap=[[1, S], [S, TL]])
            nc.sync.dma_start(norm_sb[:], norm_src)
            normB = dataclasses.replace(
                norm_sb[:], ap=[list(norm_sb[:].ap)[0], [1, TL], [0, TP]])
            # guarded norm (avoid 0/0 in Q2 = Q/M1)
            normg = cpool.tile([S, TL], dt.float32)
            nc.vector.tensor_scalar(normg[:], norm_sb[:], 1e-30, None, Alu.max)
            normgB = dataclasses.replace(
                normg[:], ap=[list(normg[:].ap)[0], [1, TL], [0, TP]])

            # M1[s,(t,tp)] = -sgp*norm/128 (exact: +-1/128 * norm)
            sgp2 = cpool.tile([S, TL * TP], dt.float32)
            nc.vector.tensor_scalar(sgp2[:], posraw[:], 0.0, 2.0, Alu.is_lt, Alu.mult)
            sgpm = cpool.tile([S, TL * TP], dt.float32)
            nc.vector.tensor_scalar(sgpm[:], sgp2[:], 1.0, 1.0 / 128, Alu.subtract, Alu.mult)
            M1 = cpool.tile([S, TL * TP], dt.float32)
            nc.vector.tensor_tensor(
                M1[:].rearrange("s (t p) -> s t p", p=TP),
                sgpm[:].rearrange("s (t p) -> s t p", p=TP),
                normgB, Alu.mult)

            # sta replicated to all partitions: [s, (t,tp)]
            starep = cpool.tile([S, TL * TP], dt.int32)
            sta_src = dataclasses.replace(
                sta_d[:].flatten(), ap=[[0, S], [1, TL * TP]])
            nc.sync.dma_start(starep[:], sta_src)

            # g path -> Q
            stamag = cpool.tile([S, TL * TP], dt.uint16)
            nc.scalar.activation(stamag[:], starep[:], Act.Abs)
            zg16 = cpool.tile([S, TL * TP], dt.uint16)
            nc.vector.tensor_tensor(zg16[:], stamag[:], p_mag16[:], Alu.bitwise_xor)
            wg = cpool.tile([S, TL * TP], dt.float32)
            nc.scalar.activation(wg[:], zg16[:], Act.Copy, bias=1.5, scale=1.0)
            eg32 = cpool.tile([S, TL * TP], dt.int32)
            nc.vector.tensor_scalar(eg32[:], wg[:].bitcast(dt.int32), 23, None,
                                    Alu.logical_shift_right)
            sag = cpool.tile([S, TL * TP], dt.bfloat16)
            nc.vector.tensor_scalar(sag[:], eg32[:], 142.0, None, Alu.subtract)
            sgxp = cpool.tile([S, TL * TP], dt.int32)
            nc.vector.tensor_tensor(sgxp[:], starep[:], posraw[:], Alu.bitwise_xor)
            sgx32 = cpool.tile([S, TL * TP], dt.int32)
            nc.vector.tensor_scalar(sgx32[:], sgxp[:], 16, 0x8000,
                                    Alu.logical_shift_right, Alu.bitwise_and)
            sgx16 = cpool.tile([S, TL * TP], dt.uint16)
            nc.vector.tensor_scalar(sgx16[:], sgx32[:], 1.0, None, Alu.mult)
            rg16 = cpool.tile([S, TL * TP], dt.uint16)
            nc.vector.tensor_tensor(rg16[:], sag[:].bitcast(dt.uint16), sgx16[:],
                                    Alu.bitwise_xor)
            t1 = cpool.tile([S, TL * TP], dt.float32)
            nc.vector.tensor_tensor(
                t1[:].rearrange("s (t p) -> s t p", p=TP),
                rg16[:].bitcast(dt.bfloat16).rearrange("s (t p) -> s t p", p=TP),
                normB, Alu.mult)
            t2 = cpool.tile([S, TL], dt.float32)
            nc.vector.tensor_reduce(t2[:].unsqueeze(2),
                                    t1[:].rearrange("s (t p) -> s t p", p=TP),
                                    axis=mybir.AxisListType.X, op=Alu.add)
            t2s = cpool.tile([S, TL], dt.float32)
            nc.vector.tensor_scalar(t2s[:], t2[:], 1.0 / 128, None, Alu.mult)
            Q = cpool.tile([S, TL * TP], dt.float32)
            nc.vector.scalar_tensor_tensor(
                Q[:].rearrange("s (t p) -> s t p", p=TP),
                t1[:].rearrange("s (t p) -> s t p", p=TP),
                1.0 / 128,
                t2s[:].unsqueeze(2).to_broadcast([S, TL, TP]),
                Alu.mult, Alu.subtract)

            cm142 = cpool.tile([S, 1], dt.float32)
            nc.vector.memset(cm142[:], -142.0)

            # Q2 = Q/M1 (norm cancels algebraically; normg guards 0/0),
            # bf16 copies for the wide DVE p5 path
            recM1 = cpool.tile([S, TL * TP], dt.float32)
            nc.vector.reciprocal(recM1[:], M1[:])
            Q2 = cpool.tile([S, TL * TP], dt.float32)
            nc.vector.tensor_tensor(Q2[:], Q[:], recM1[:], Alu.mult)
            Q2b = cpool.tile([S, TL * TP], dt.bfloat16)
            nc.vector.tensor_scalar(Q2b[:], Q2[:], 0.0, None, Alu.add)
            M1b = cpool.tile([S, TL * TP], dt.bfloat16)
            nc.vector.tensor_scalar(M1b[:], M1[:], 0.0, None, Alu.add)

            # ---------------- main loop over t ----------------
            ND = TP - ns  # tp lanes on the DVE wide path
            for t in range(TL):
                cncR = wpool.tile([S, FD], dt.uint16, tag="cncR")
                scR = wpool.tile([S, FD], dt.bfloat16, tag="scR")
                nc.sync.dma_start(cncR[:], _rep128(d_cmag[t:t + 1, :]))
                nc.sync.dma_start(scR[:], _rep128(d_csgn[t:t + 1, :]))

                # p1: z = cmag ^ pmag (pmag bcast along c, tp innermost)
                z16 = wpool.tile([S, FD], dt.uint16, tag="z16")
                pm_b = _sub(p_mag16[:], t * TP, [[0, C], [1, TP]])
                nc.vector.tensor_tensor(
                    z16[:].rearrange("s (c p) -> s c p", p=TP),
                    cncR[:].rearrange("s (c p) -> s c p", p=TP),
                    pm_b, Alu.bitwise_xor)

                # p2: w = fp32(z) + 1.5 (exact frexp trick)
                w1 = wpool.tile([S, FD], dt.float32, tag="w1")
                nc.scalar.activation(w1[:], z16[:], Act.Copy, bias=1.5, scale=1.0)

                # p3a: e16 = hi16(bits(w)) >> 7 (biased exponent, u16 contiguous)
                e16 = wpool.tile([S, FD], dt.uint16, tag="e16")
                w_hi = _sub(w1[:].bitcast(dt.uint16), 1, [[2, FD]])
                nc.vector.tensor_scalar(e16[:], w_hi, 7, None,
                                        Alu.logical_shift_right)

                # p3b+p4 fused: r = (e16 - 142) * sgn  (= sgc*(e-16), bf16)
                r16 = wpool.tile([S, FD], dt.bfloat16, tag="r16")
                nc.vector.scalar_tensor_tensor(r16[:], e16[:], 142.0, scR[:],
                                               Alu.subtract, Alu.mult)

                out_sb = wpool.tile([S, C, TP], odt, tag="out")
                out_flat = out_sb[:].rearrange("s c p -> s (c p)")

                # p5 ACT lanes [0, ns): out = M1*r + Q, exact fp32 affine
                for tp in range(ns):
                    rsl = _sub(r16[:], tp, [[TP, C]])
                    dsl = _sub(out_flat, tp, [[TP, C]])
                    nc.scalar.activation(dsl, rsl, Act.Identity,
                                         bias=Q[:, t * TP + tp: t * TP + tp + 1],
                                         scale=M1[:, t * TP + tp: t * TP + tp + 1])

                # p5 DVE lanes [ns, 8): r2 = r + Q2 ; out = r2 * M1
                if ND:
                    r2 = wpool.tile([S, C * ND], dt.bfloat16, tag="r2")
                    r16s = _sub(r16[:], ns, [[TP, C], [1, ND]])
                    q2_b = _sub(Q2b[:], t * TP + ns, [[0, C], [1, ND]])
                    nc.vector.tensor_tensor(
                        r2[:].rearrange("s (c p) -> s c p", p=ND), r16s, q2_b,
                        Alu.add)
                    m1_b = _sub(M1b[:], t * TP + ns, [[0, C], [1, ND]])
                    outs = _sub(out_flat, ns, [[TP, C], [1, ND]])
                    nc.vector.tensor_tensor(
                        outs, r2[:].rearrange("s (c p) -> s c p", p=ND), m1_b,
                        Alu.mult)

                nc.sync.dma_start(ct_d[t], out_sb[:])

    nc.compile()
    return nc


_NC_CACHE = None


def _get_nc():
    global _NC_CACHE
    if _NC_CACHE is None:
        _NC_CACHE = build_nc()
    return _NC_CACHE


def kernel(sta_loc, pos_loc, cnc_loc, eu_norm):
    nc = _get_nc()

    sta_loc = np.ascontiguousarray(np.asarray(sta_loc, dtype=np.int32))
    pos_loc = np.ascontiguousarray(np.asarray(pos_loc, dtype=np.int32))
    cnc_loc = np.ascontiguousarray(np.asarray(cnc_loc, dtype=np.int32))
    eu_norm = np.ascontiguousarray(np.asarray(eu_norm, dtype=np.float32))

    in_maps = []
    for c in range(NCORES):
        lo, hi = c * TL, (c + 1) * TL
        in_maps.append({
            "sta_loc": sta_loc[lo:hi],
            "pos_loc": pos_loc[lo:hi],
            "cnc_loc": cnc_loc[lo:hi],
            "eu_norm": eu_norm[lo:hi],
        })
    res = run_bass_kernel_spmd(nc, in_maps, core_ids=list(range(NCORES)))
    out = np.concatenate([r["ct"] for r in res.results], axis=0)
    return np.asarray(out, dtype=np.float32)


def run_traced(inputs, trace=True):
    """For test.py: run with NTFF tracing, return (out, BassKernelResults)."""
    nc = _get_nc()
    in_maps = []
    for c in range(NCORES):
        lo, hi = c * TL, (c + 1) * TL
        in_maps.append({k: np.ascontiguousarray(v[lo:hi]) for k, v in inputs.items()})
    res = run_bass_kernel_spmd(nc, in_maps, core_ids=list(range(NCORES)), trace=trace)
    out = np.concatenate([r["ct"] for r in res.results], axis=0)
    return np.asarray(out, dtype=np.float32), res


# revision 42
# speedup vs baseline: 1.0321x; 1.0321x over previous
"""Trainium2 Bass kernel for nn_CritiGraph.

Math (vs the fp32 reference):
  dist(c1,c2,n) = sg * (1 - e/16) * n,  sg = sign(c1)*sign(c2),
  e = frexp_exp(|c1|^|c2| + 1) = bexp(float(|c1|^|c2|) + 1.5) - 126.
  ct[t,s,c,tp] = Q[t,s,tp] + M1[t,s,tp] * R[t,s,c,tp]
  where R = sgc * (e-16) (cnc sign applied via bf16 sign-bit xor),
        M1 = -sgp * norm / 128  (pos sign + /TP/16 folded),
        Q  = (sum_tp g - g)/8,  g = cos_sta_pos.

v2 layout: everything in natural (c,tp) order, S=128 on partitions.
Main loop per t (all full-width 2056-elem ops):
  p1  DVE TT : z16 = cmagR ^ pmag[s,tp] (bcast along c, stride-0 free dim)
  p2  ACT    : w = fp32(z16) + 1.5
  p3  DVE TS : sa = bf16((bits(w) >> 23) - 142)       [= e - 16, signed]
  p4  DVE TT : r16 = bits(sa) ^ sgn_mask (0x8000/0)
  p5  split by tp lane:
      lanes [0,NS):  ACT affine  out = M1*r + Q   (per-partition scale/bias)
      lanes [NS,8):  DVE TT r2 = r + Q2 (Q2 = Q/M1, bcast along c)
                     DVE TT out = r2 * M1 (bcast along c)
Sharding: T=128 rows split across 8 cores (16 rows each). Output gathered
on host (and upcast to fp32 if OUT_DT is bf16).
"""
import dataclasses
import numpy as np

import concourse.bass as bass
import concourse.mybir as mybir
from concourse import tile, bacc
from concourse.bass_utils import run_bass_kernel_spmd

dt = mybir.dt
Alu = mybir.AluOpType
Act = mybir.ActivationFunctionType

T, S, TP, C = 128, 128, 8, 257
NCORES = 8
TL = T // NCORES          # 16 t-rows per core
FD = C * TP               # 2056 free width, (c,tp) natural order

NS = 4                    # tp lanes [0,NS) handled by ACT affine slices
OUT_DT = "bf16"           # 'f32' | 'bf16' (bf16 is upcast to f32 on host)


def _rep128(ap_row):
    """DRAM row AP -> same row broadcast to 128 partitions (stride-0)."""
    return dataclasses.replace(ap_row, ap=[[0, 128]] + list(ap_row.ap)[1:])


def _sub(ap_flat, off_elems, dims):
    """Offset a flat [P, N] AP by off_elems and install custom free dims."""
    s = ap_flat[:, off_elems:off_elems + 1]
    return dataclasses.replace(s, ap=[list(s.ap)[0]] + [list(d) for d in dims])


def build_nc(ns=None, out_dt=None):
    ns = NS if ns is None else ns
    out_dt = OUT_DT if out_dt is None else out_dt
    odt = dt.float32 if out_dt == "f32" else dt.bfloat16
    nc = bacc.Bacc("TRN2", target_bir_lowering=False, debug=False)

    ND = TP - ns
    sta_d = nc.dram_tensor("sta_loc", [TL, TP], dt.int32, kind="ExternalInput")
    pos_d = nc.dram_tensor("pos_loc", [TL, S, TP], dt.int32, kind="ExternalInput")
    cnc_d = nc.dram_tensor("cnc_loc", [TL, C, TP], dt.int32, kind="ExternalInput")
    norm_d = nc.dram_tensor("eu_norm", [TL, S], dt.float32, kind="ExternalInput")
    # output split by tp lane: ACT-written lanes [0,NS) in fp32,
    # DVE-written lanes [NS,8) in bf16; stitched on host.
    cta_d = (nc.dram_tensor("ct_a", [TL, S, C, ns], dt.float32,
                            kind="ExternalOutput") if ns else None)
    ctb_d = (nc.dram_tensor("ct_b", [TL, S, C, ND], odt,
                            kind="ExternalOutput") if ND else None)

    with tile.TileContext(nc) as tc:
        with (
            tc.tile_pool(name="const", bufs=1) as cpool,
            tc.tile_pool(name="work", bufs=4) as wpool,
            tc.tile_pool(name="dram", bufs=1, space="DRAM") as dpool,
        ):
            # ---------------- preprocessing (small) ----------------
            # cnc -> (c,tp) magnitude / sign-mask planes, staged to DRAM
            cncraw = cpool.tile([TL, FD], dt.int32)
            nc.sync.dma_start(cncraw[:], cnc_d[:].rearrange("t c p -> t (c p)"))
            c_mag16 = cpool.tile([TL, FD], dt.uint16)
            c_sgn16 = cpool.tile([TL, FD], dt.uint16)
            nc.scalar.activation(c_mag16[:], cncraw[:], Act.Abs)
            # sign as bf16 sign-bit mask: 0x8000 if cnc < 0 else 0
            nc.vector.tensor_scalar(c_sgn16[:], cncraw[:], 0.0, 32768.0,
                                    Alu.is_lt, Alu.mult)
            d_cmag = dpool.tile([TL, FD], dt.uint16)
            d_csgn = dpool.tile([TL, FD], dt.uint16)
            nc.sync.dma_start(d_cmag[:], c_mag16[:])
            nc.sync.dma_start(d_csgn[:], c_sgn16[:])

            # pos: [s, (t,tp)] in one strided DMA
            posraw = cpool.tile([S, TL * TP], dt.int32)
            pos_src = dataclasses.replace(
                pos_d[:].rearrange("t s p -> t (s p)").flatten(),
                ap=[[TP, S], [S * TP, TL], [1, TP]])
            nc.sync.dma_start(
                posraw[:].rearrange("s (t p) -> s t p", p=TP), pos_src)
            p_mag16 = cpool.tile([S, TL * TP], dt.uint16)
            nc.scalar.activation(p_mag16[:], posraw[:], Act.Abs)

            # norm[s, t] via strided DMA (4B gather, 8KB once)
            norm_sb = cpool.tile([S, TL], dt.float32)
            norm_src = dataclasses.replace(
                norm_d[:].flatten(), ap=[[1, S], [S, TL]])
            nc.sync.dma_start(norm_sb[:], norm_src)
            normB = dataclasses.replace(
                norm_sb[:], ap=[list(norm_sb[:].ap)[0], [1, TL], [0, TP]])
            # guarded norm (avoid 0/0 in Q2 = Q/M1)
            normg = cpool.tile([S, TL], dt.float32)
            nc.vector.tensor_scalar(normg[:], norm_sb[:], 1e-30, None, Alu.max)
            normgB = dataclasses.replace(
                normg[:], ap=[list(normg[:].ap)[0], [1, TL], [0, TP]])

            # M1[s,(t,tp)] = -sgp*norm/128 (exact: +-1/128 * norm)
            sgp2 = cpool.tile([S, TL * TP], dt.float32)
            nc.vector.tensor_scalar(sgp2[:], posraw[:], 0.0, 2.0, Alu.is_lt, Alu.mult)
            sgpm = cpool.tile([S, TL * TP], dt.float32)
            nc.vector.tensor_scalar(sgpm[:], sgp2[:], 1.0, 1.0 / 128, Alu.subtract, Alu.mult)
            M1 = cpool.tile([S, TL * TP], dt.float32)
            nc.vector.tensor_tensor(
                M1[:].rearrange("s (t p) -> s t p", p=TP),
                sgpm[:].rearrange("s (t p) -> s t p", p=TP),
                normgB, Alu.mult)

            # sta replicated to all partitions: [s, (t,tp)]
            starep = cpool.tile([S, TL * TP], dt.int32)
            sta_src = dataclasses.replace(
                sta_d[:].flatten(), ap=[[0, S], [1, TL * TP]])
            nc.sync.dma_start(starep[:], sta_src)

            # g path -> Q
            stamag = cpool.tile([S, TL * TP], dt.uint16)
            nc.scalar.activation(stamag[:], starep[:], Act.Abs)
            zg16 = cpool.tile([S, TL * TP], dt.uint16)
            nc.vector.tensor_tensor(zg16[:], stamag[:], p_mag16[:], Alu.bitwise_xor)
            wg = cpool.tile([S, TL * TP], dt.float32)
            nc.scalar.activation(wg[:], zg16[:], Act.Copy, bias=1.5, scale=1.0)
            eg32 = cpool.tile([S, TL * TP], dt.int32)
            nc.vector.tensor_scalar(eg32[:], wg[:].bitcast(dt.int32), 23, None,
                                    Alu.logical_shift_right)
            sag = cpool.tile([S, TL * TP], dt.bfloat16)
            nc.vector.tensor_scalar(sag[:], eg32[:], 142.0, None, Alu.subtract)
            sgxp = cpool.tile([S, TL * TP], dt.int32)
            nc.vector.tensor_tensor(sgxp[:], starep[:], posraw[:], Alu.bitwise_xor)
            sgx32 = cpool.tile([S, TL * TP], dt.int32)
            nc.vector.tensor_scalar(sgx32[:], sgxp[:], 16, 0x8000,
                                    Alu.logical_shift_right, Alu.bitwise_and)
            sgx16 = cpool.tile([S, TL * TP], dt.uint16)
            nc.vector.tensor_scalar(sgx16[:], sgx32[:], 1.0, None, Alu.mult)
            rg16 = cpool.tile([S, TL * TP], dt.uint16)
            nc.vector.tensor_tensor(rg16[:], sag[:].bitcast(dt.uint16), sgx16[:],
                                    Alu.bitwise_xor)
            t1 = cpool.tile([S, TL * TP], dt.float32)
            nc.vector.tensor_tensor(
                t1[:].rearrange("s (t p) -> s t p", p=TP),
                rg16[:].bitcast(dt.bfloat16).rearrange("s (t p) -> s t p", p=TP),
                normB, Alu.mult)
            t2 = cpool.tile([S, TL], dt.float32)
            nc.vector.tensor_reduce(t2[:].unsqueeze(2),
                                    t1[:].rearrange("s (t p) -> s t p", p=TP),
                                    axis=mybir.AxisListType.X, op=Alu.add)
            t2s = cpool.tile([S, TL], dt.float32)
            nc.vector.tensor_scalar(t2s[:], t2[:], 1.0 / 128, None, Alu.mult)
            Q = cpool.tile([S, TL * TP], dt.float32)
            nc.vector.scalar_tensor_tensor(
                Q[:].rearrange("s (t p) -> s t p", p=TP),
                t1[:].rearrange("s (t p) -> s t p", p=TP),
                1.0 / 128,
                t2s[:].unsqueeze(2).to_broadcast([S, TL, TP]),
                Alu.mult, Alu.subtract)

            # Q2 = Q/M1 (norm cancels algebraically; normg guards 0/0),
            # bf16 copies for the wide DVE p5 path
            recM1 = cpool.tile([S, TL * TP], dt.float32)
            nc.vector.reciprocal(recM1[:], M1[:])
            Q2 = cpool.tile([S, TL * TP], dt.float32)
            nc.vector.tensor_tensor(Q2[:], Q[:], recM1[:], Alu.mult)
            Q2b = cpool.tile([S, TL * TP], dt.bfloat16)
            nc.vector.tensor_scalar(Q2b[:], Q2[:], 0.0, None, Alu.add)
            M1b = cpool.tile([S, TL * TP], dt.bfloat16)
            nc.vector.tensor_scalar(M1b[:], M1[:], 0.0, None, Alu.add)

            # ---------------- main loop over t ----------------
            for t in range(TL):
                cncR = wpool.tile([S, FD], dt.uint16, tag="cncR")
                scR = wpool.tile([S, FD], dt.uint16, tag="scR")
                nc.sync.dma_start(cncR[:], _rep128(d_cmag[t:t + 1, :]))
                nc.sync.dma_start(scR[:], _rep128(d_csgn[t:t + 1, :]))

                # p1: z = cmag ^ pmag (pmag bcast along c, tp innermost)
                z16 = wpool.tile([S, FD], dt.uint16, tag="z16")
                pm_b = _sub(p_mag16[:], t * TP, [[0, C], [1, TP]])
                nc.vector.tensor_tensor(
                    z16[:].rearrange("s (c p) -> s c p", p=TP),
                    cncR[:].rearrange("s (c p) -> s c p", p=TP),
                    pm_b, Alu.bitwise_xor)

                # p2: w = fp32(z) + 1.5 (exact frexp trick)
                w1 = wpool.tile([S, FD], dt.float32, tag="w1")
                nc.scalar.activation(w1[:], z16[:], Act.Copy, bias=1.5, scale=1.0)

                # p3a: e16 = hi16(bits(w)) >> 7 (biased exponent, u16 contiguous)
                e16 = wpool.tile([S, FD], dt.uint16, tag="e16")
                w_hi = _sub(w1[:].bitcast(dt.uint16), 1, [[2, FD]])
                nc.vector.tensor_scalar(e16[:], w_hi, 7, None,
                                        Alu.logical_shift_right)

                # p3b: sa = bf16(e16 - 142) = e - 16 (2-byte 2x fast path)
                sa = wpool.tile([S, FD], dt.bfloat16, tag="sa")
                nc.vector.tensor_scalar(sa[:], e16[:], 142.0, None, Alu.subtract)

                # p4: r = sa ^ sign-mask (contiguous u16)
                r16 = wpool.tile([S, FD], dt.bfloat16, tag="r16")
                nc.vector.tensor_tensor(r16[:].bitcast(dt.uint16),
                                        sa[:].bitcast(dt.uint16), scR[:],
                                        Alu.bitwise_xor)

                # p5 ACT lanes [0, ns): out_a = M1*r + Q, exact fp32 affine
                if ns:
                    out_a = wpool.tile([S, C, ns], dt.float32, tag="outa")
                    oa_flat = out_a[:].rearrange("s c p -> s (c p)")
                    for tp in range(ns):
                        rsl = _sub(r16[:], tp, [[TP, C]])
                        dsl = _sub(oa_flat, tp, [[ns, C]])
                        nc.scalar.activation(
                            dsl, rsl, Act.Identity,
                            bias=Q[:, t * TP + tp: t * TP + tp + 1],
                            scale=M1[:, t * TP + tp: t * TP + tp + 1])
                    nc.sync.dma_start(cta_d[t], out_a[:])

                # p5 DVE lanes [ns, 8): r2 = r + Q2 ; out_b = r2 * M1 (bf16)
                if ND:
                    out_b = wpool.tile([S, C * ND], odt, tag="outb")
                    r2 = wpool.tile([S, C * ND], dt.bfloat16, tag="r2")
                    r16s = _sub(r16[:], ns, [[TP, C], [1, ND]])
                    q2_b = _sub(Q2b[:], t * TP + ns, [[0, C], [1, ND]])
                    nc.vector.tensor_tensor(
                        r2[:].rearrange("s (c p) -> s c p", p=ND), r16s, q2_b,
                        Alu.add)
                    m1_b = _sub(M1b[:], t * TP + ns, [[0, C], [1, ND]])
                    nc.vector.tensor_tensor(
                        out_b[:].rearrange("s (c p) -> s c p", p=ND),
                        r2[:].rearrange("s (c p) -> s c p", p=ND), m1_b,
                        Alu.mult)
                    nc.sync.dma_start(ctb_d[t], out_b[:])

    nc.compile()
    return nc


_NC_CACHE = None


def _get_nc():
    global _NC_CACHE
    if _NC_CACHE is None:
        _NC_CACHE = build_nc()
    return _NC_CACHE


def kernel(sta_loc, pos_loc, cnc_loc, eu_norm):
    nc = _get_nc()

    sta_loc = np.ascontiguousarray(np.asarray(sta_loc, dtype=np.int32))
    pos_loc = np.ascontiguousarray(np.asarray(pos_loc, dtype=np.int32))
    cnc_loc = np.ascontiguousarray(np.asarray(cnc_loc, dtype=np.int32))
    eu_norm = np.ascontiguousarray(np.asarray(eu_norm, dtype=np.float32))

    in_maps = []
    for c in range(NCORES):
        lo, hi = c * TL, (c + 1) * TL
        in_maps.append({
            "sta_loc": sta_loc[lo:hi],
            "pos_loc": pos_loc[lo:hi],
            "cnc_loc": cnc_loc[lo:hi],
            "eu_norm": eu_norm[lo:hi],
        })
    res = run_bass_kernel_spmd(nc, in_maps, core_ids=list(range(NCORES)))
    return _stitch(res.results)


def _stitch(results):
    """Merge per-core ct_a (fp32 lanes [0,NS)) + ct_b (bf16 lanes [NS,8))
    into the full (T,S,C,TP) fp32 output."""
    out = np.empty((T, S, C, TP), dtype=np.float32)
    for c, r in enumerate(results):
        lo, hi = c * TL, (c + 1) * TL
        if "ct_a" in r:
            out[lo:hi, :, :, :NS] = r["ct_a"]
        if "ct_b" in r:
            out[lo:hi, :, :, NS:] = r["ct_b"].astype(np.float32)
    return out


def run_traced(inputs, trace=True):
    """For test.py: run with NTFF tracing, return (out, BassKernelResults)."""
    nc = _get_nc()
    in_maps = []
    for c in range(NCORES):
        lo, hi = c * TL, (c + 1) * TL
        in_maps.append({k: np.ascontiguousarray(v[lo:hi]) for k, v in inputs.items()})
    res = run_bass_kernel_spmd(nc, in_maps, core_ids=list(range(NCORES)), trace=trace)
    return _stitch(res.results), res
